# revision 1
# baseline (speedup 1.0000x reference)
"""Attention4D (dense_transformer) — 8-core row-sharded implementation.

Strategy: the attention-score rows (query pixels, N = 56*56 = 3136) are
independent through scores -> talking-head-1 -> softmax -> talking-head-2
-> att@v -> relu -> proj, because the talking-head 1x1 convs mix HEADS,
not pixels.  So we shard the N axis across the 8 cores (392 rows each):
each core computes k/v in full (cheap) and only its block of q rows, and
no collective at all is needed — just the final gather.  Falls back to a
pure-numpy implementation if 8 accelerator devices are unavailable.
"""
import os
import numpy as np

NUM_HEAD, DIM, DIM_K = 8, 128, 16
D = 64
DH = NUM_HEAD * D          # 512
B, H, W = 2, 56, 56
N = H * W                  # 3136
NCORES = 8
R = N // NCORES            # 392 query rows per core
SCALE = DIM_K ** (-0.5)


def _kernel_numpy(x, wq, bq, wk, bk, wv, bv, w_vl, b_vl,
                  w_th1, b_th1, w_th2, b_th2, w_proj, b_proj):
    f = np.float32
    xf = x.reshape(B, DIM, N)                                   # [B,C,N]
    q = (np.einsum('oc,bcn->bon', wq, xf) + bq[None, :, None]).reshape(B, NUM_HEAD, DIM_K, N)
    q = np.ascontiguousarray(q.transpose(0, 1, 3, 2))           # [B,nh,N,dk]
    k = (np.einsum('oc,bcn->bon', wk, xf) + bk[None, :, None]).reshape(B, NUM_HEAD, DIM_K, N)
    v = (np.einsum('oc,bcn->bon', wv, xf) + bv[None, :, None]).reshape(B, DH, H, W)
    vp = np.pad(v, ((0, 0), (0, 0), (1, 1), (1, 1)))
    v2 = np.broadcast_to(b_vl[None, :, None, None], v.shape).astype(f).copy()
    for di in range(3):
        for dj in range(3):
            v2 += vp[:, :, di:di + H, dj:dj + W] * w_vl[None, :, 0, di, dj][:, :, None, None]
    v2 = v2.reshape(B, NUM_HEAD, D, N).transpose(0, 1, 3, 2)    # [B,nh,N,d]
    att = np.einsum('bhnd,bhdm->bhnm', q, k, optimize=True) * f(SCALE)
    att = np.einsum('bhnm,gh->bgnm', att, w_th1, optimize=True) + b_th1[None, :, None, None]
    att -= att.max(axis=-1, keepdims=True)
    np.exp(att, out=att)
    att /= att.sum(axis=-1, keepdims=True)
    att = np.einsum('bhnm,gh->bgnm', att, w_th2, optimize=True) + b_th2[None, :, None, None]
    net = np.einsum('bhnm,bhmd->bhnd', att, v2, optimize=True)  # [B,nh,N,d]
    net = net.transpose(0, 1, 3, 2).reshape(B, DH, N)
    np.maximum(net, 0, out=net)
    out = np.einsum('oc,bcn->bon', w_proj, net) + b_proj[None, :, None]
    return np.ascontiguousarray(out.reshape(B, DIM, H, W), dtype=np.float32)


def _kernel_neuron(x, wq, bq, wk, bk, wv, bv, w_vl, b_vl,
                   w_th1, b_th1, w_th2, b_th2, w_proj, b_proj):
    import jax
    import jax.numpy as jnp
    from jax import lax

    devs = jax.devices()
    if len(devs) < NCORES or devs[0].platform == 'cpu':
        raise RuntimeError('need 8 accelerator devices, have %r' % (devs,))

    xj = jnp.asarray(x)
    cst = {n: jnp.asarray(a) for n, a in dict(
        wq=wq, bq=bq, wk=wk, bk=bk, wv=wv, bv=bv, w_vl=w_vl, b_vl=b_vl,
        w_th1=w_th1, b_th1=b_th1, w_th2=w_th2, b_th2=b_th2,
        w_proj=w_proj, b_proj=b_proj).items()}

    def shard_fn(row0):
        xf = xj.reshape(B, DIM, N)
        q = (jnp.einsum('oc,bcn->bon', cst['wq'], xf)
             + cst['bq'][None, :, None]).reshape(B, NUM_HEAD, DIM_K, N)
        q = q.transpose(0, 1, 3, 2)                              # [B,nh,N,dk]
        qr = lax.dynamic_slice_in_dim(q, row0, R, axis=2)        # [B,nh,R,dk]
        k = (jnp.einsum('oc,bcn->bon', cst['wk'], xf)
             + cst['bk'][None, :, None]).reshape(B, NUM_HEAD, DIM_K, N)
        v = (jnp.einsum('oc,bcn->bon', cst['wv'], xf)
             + cst['bv'][None, :, None]).reshape(B, DH, H, W)
        vp = jnp.pad(v, ((0, 0), (0, 0), (1, 1), (1, 1)))
        v2 = jnp.broadcast_to(cst['b_vl'][None, :, None, None], v.shape)
        for di in range(3):
            for dj in range(3):
                v2 = v2 + vp[:, :, di:di + H, dj:dj + W] * \
                    cst['w_vl'][None, :, 0, di, dj][:, :, None, None]
        v2 = v2.reshape(B, NUM_HEAD, D, N).transpose(0, 1, 3, 2)  # [B,nh,N,d]
        att = jnp.einsum('bhnd,bhdm->bhnm', qr, k) * SCALE        # [B,nh,R,N]
        att = jnp.einsum('bhnm,gh->bgnm', att, cst['w_th1']) + cst['b_th1'][None, :, None, None]
        att = jax.nn.softmax(att, axis=-1)
        att = jnp.einsum('bhnm,gh->bgnm', att, cst['w_th2']) + cst['b_th2'][None, :, None, None]
        net = jnp.einsum('bhnm,bhmd->bhnd', att, v2)              # [B,nh,R,d]
        net = net.transpose(0, 1, 3, 2).reshape(B, DH, R)
        net = jax.nn.relu(net)
        return jnp.einsum('oc,bcn->bon', cst['w_proj'], net) + cst['b_proj'][None, :, None]

    row0s = jnp.arange(NCORES, dtype=jnp.int32) * R
    out = jax.pmap(shard_fn, devices=devs[:NCORES])(row0s)        # [8,B,DIM,R]
    out = np.asarray(out)
    out = out.transpose(1, 2, 0, 3).reshape(B, DIM, N)
    return np.ascontiguousarray(out.reshape(B, DIM, H, W), dtype=np.float32)


def kernel(**inputs):
    inputs = {n: np.asarray(a, dtype=np.float32) for n, a in inputs.items()}
    if os.environ.get('KERNEL_FORCE_NUMPY'):
        return _kernel_numpy(**inputs)
    timeout = int(os.environ.get('KERNEL_NEURON_TIMEOUT', '300'))
    import signal
    old = None
    try:
        def _alarm(signum, frame):
            raise TimeoutError('neuron path timed out')
        old = signal.signal(signal.SIGALRM, _alarm)
        signal.alarm(timeout)
    except (ValueError, OSError):
        old = None  # not in main thread; run unguarded
    try:
        return _kernel_neuron(**inputs)
    except BaseException:
        return _kernel_numpy(**inputs)
    finally:
        try:
            signal.alarm(0)
            if old is not None:
                signal.signal(signal.SIGALRM, old)
        except (ValueError, OSError):
            pass



# revision 2
# speedup vs baseline: 308.7698x; 308.7698x over previous
"""Attention4D (talking-heads attention) — Trainium2 Bass kernel, 8-core SPMD.

Self-contained: builds a Bass/Tile kernel (via concourse from /opt/trn_rl_repo),
shards the 3136 query pixels across 8 NeuronCores, and runs via
bass_utils.run_bass_kernel_spmd.  Falls back to a pure-numpy implementation
if the Neuron stack is unavailable.
"""
import os
import sys

for _p in ('/opt/trn_rl_repo',):
    if _p not in sys.path and os.path.isdir(_p):
        sys.path.insert(0, _p)

"""Bass/Tile Trainium2 kernel for Attention4D (talking-heads attention).

Sharding: N = 56*56 = 3136 query pixels split across 8 cores (392 each).
Each core computes full k and (1/8 of) v2; v2^T is AllGathered.

Math per batch b, per core (R = 392 query rows, padded to 400 = 25 slices
of 16):
  scores+th1:  att1[(g,ns), m] = sum_hd Qhat_j[hd,(g,ns)] * k[hd, m]
               where Qhat_j[hd,(g,ns)] = SCALE * w1[g, head(hd)] * q[hd, 16j+ns]
  softmax:     E = exp(att1 + b_th1[g]); s[(g,ns)] = sum_m E  (ACT accum_out)
  th2+norm:    A2[(g,ns), m] = sum_(h,i) W2kron[(h,i),(g,ns)]/s[(h,i)] * E[(h,i), m]
               (+ b_th2[g] added at eviction)
  transpose:   A2T[m, (j, (g,ns))] via DMA transpose
  av:          O^T[(g,d), n] = sum_m v2T[m, (g,d)] * A2T[m, (j,ns)-for-g]
  out:         proj(relu(O^T)) + b_proj
"""
import numpy as np
import ml_dtypes
from contextlib import ExitStack

B = 2
C = 128
NH = 8
DK = 16
D = 64
DH = 512
H = W = 56
N = H * W            # 3136
NCORES = 8
R = N // NCORES      # 392 rows per core
RP = 400             # padded (25 slices of 16)
NJ = 25
YS = 7               # y-rows per core (R = 7*56)
XVLEN = 9 * 56       # 504: y rows with halo
SCALE = DK ** -0.5

BF16 = ml_dtypes.bfloat16


def host_inputs(x, wq, bq, wk, bk, wv, bv, w_vl, b_vl,
                w_th1, b_th1, w_th2, b_th2, w_proj, b_proj):
    """Build per-core input maps (numpy only, cheap)."""
    f32 = np.float32
    xf = np.ascontiguousarray(x.reshape(B, C, N), dtype=f32)

    gidx = np.arange(128) // 16   # g for partition (g,ns)
    hidx = np.arange(128) // 16   # h for partition (h,i)

    # W1F[hd, (g,ns)] = SCALE * w_th1[g, head(hd)]  (ns-independent)
    W1F = (SCALE * w_th1[gidx][:, np.arange(128) // 16].T).astype(BF16)
    # Actually: W1F[hd, col] with col=(g,ns): value = SCALE*w_th1[g, hd//16]
    W1F = np.empty((128, 128), dtype=np.float32)
    for hd in range(128):
        for col in range(128):
            W1F[hd, col] = SCALE * w_th1[col // 16, hd // 16]
    W1F = W1F.astype(BF16)

    # W2kron[(h,i), (g,ns)] = w_th2[g,h] * (i==ns)
    W2K = np.zeros((128, 128), dtype=np.float32)
    for row in range(128):
        h, i = row // 16, row % 16
        for col in range(128):
            g, ns = col // 16, col % 16
            if i == ns:
                W2K[row, col] = w_th2[g, h]
    W2K = W2K.astype(BF16)

    bth1P = b_th1[gidx].reshape(128, 1).astype(f32)
    bth2P = b_th2[gidx].reshape(128, 1).astype(f32)

    # dwconv tap weights: wvlP[ch_part, ct, tap], taps (dy,dx) row-major
    wvl = w_vl.reshape(DH, 9)
    wvlP = np.empty((128, 4, 9), dtype=f32)
    bvlP = np.empty((128, 4), dtype=f32)
    bvP = np.empty((128, 4), dtype=f32)
    for ct in range(4):
        wvlP[:, ct, :] = wvl[128 * ct:128 * (ct + 1), :]
        bvlP[:, ct] = b_vl[128 * ct:128 * (ct + 1)]
        bvP[:, ct] = bv[128 * ct:128 * (ct + 1)]

    # wprojP[p, r, c] = w_proj[c, ch] with ch = (2r + p//64)*64 + p%64
    wprojP = np.empty((128, 4, 128), dtype=np.float32)
    for r in range(4):
        for p in range(128):
            ch = (2 * r + p // 64) * 64 + (p % 64)
            wprojP[p, r, :] = w_proj[:, ch]
    wprojP = wprojP.astype(BF16)

    shared = dict(
        wkt=np.ascontiguousarray(wk.T).astype(BF16),
        wqt=np.ascontiguousarray(wq.T).astype(BF16),
        wvt=np.ascontiguousarray(wv.T).astype(BF16),
        w1f=W1F, w2k=W2K,
        bth1=bth1P, bth2=bth2P,
        bk=bk.reshape(128, 1).astype(f32),
        bq=bq.reshape(128, 1).astype(f32),
        bv=bvP, wvl=wvlP, bvl=bvlP,
        wpp=wprojP,
        bproj=b_proj.reshape(128, 1).astype(f32),
        ident=np.eye(128, dtype=np.float32).astype(BF16),
    )

    in_maps = []
    for c in range(NCORES):
        r0 = c * R
        xq = np.zeros((B, C, RP), dtype=f32)
        xq[:, :, :R] = xf[:, :, r0:r0 + R]
        # v halo: y rows [7c-1, 7c+8)
        xv = np.zeros((B, C, XVLEN), dtype=f32)
        y0 = 7 * c - 1
        vmask = np.zeros((128, 9), dtype=f32)
        for yy in range(9):
            y = y0 + yy
            if 0 <= y < H:
                xv[:, :, yy * 56:(yy + 1) * 56] = xf[:, :, y * 56:(y + 1) * 56]
                vmask[:, yy] = 1.0
        m = dict(shared)
        m['x'] = xf
        m['xq'] = xq
        m['xv'] = xv
        m['vmask'] = vmask.astype(BF16)
        in_maps.append(m)
    return in_maps


def build_nc():
    import concourse.bass as bass
    import concourse.tile as tile
    import concourse.mybir as mybir
    from concourse import bacc

    dt = mybir.dt
    f32 = dt.float32
    bf16 = dt.bfloat16
    AF = mybir.ActivationFunctionType
    ALU = mybir.AluOpType

    nc = bacc.Bacc('TRN2', target_bir_lowering=False)

    def din(name, shape, dtype=f32):
        return nc.dram_tensor(name, shape, dtype, kind="ExternalInput")

    X = din('x', [B, C, N])
    XQ = din('xq', [B, C, RP])
    XV = din('xv', [B, C, XVLEN])
    WKT = din('wkt', [128, 128], bf16)
    WQT = din('wqt', [128, 128], bf16)
    WVT = din('wvt', [128, 512], bf16)
    W1F = din('w1f', [128, 128], bf16)
    W2K = din('w2k', [128, 128], bf16)
    BTH1 = din('bth1', [128, 1])
    BTH2 = din('bth2', [128, 1])
    BK = din('bk', [128, 1])
    BQ = din('bq', [128, 1])
    BV = din('bv', [128, 4])
    WVL = din('wvl', [128, 4, 9])
    BVL = din('bvl', [128, 4])
    WPP = din('wpp', [128, 4, 128], bf16)
    BPROJ = din('bproj', [128, 1])
    IDENT = din('ident', [128, 128], bf16)
    VMASK = din('vmask', [128, 9], bf16)

    OUT = nc.dram_tensor('out', [B, C, R], f32, kind="ExternalOutput")

    # collective staging (internal DRAM)
    V2TL = [nc.dram_tensor(f'v2tl{b}', [R, DH], bf16) for b in range(B)]
    V2TG = [nc.dram_tensor(f'v2tg{b}', [N, DH], bf16, addr_space="Shared")
            for b in range(B)]

    def fap(t, offset, dims):
        """Custom AP: partition dim from t's AP, free dims as given."""
        base = t[:] if not isinstance(t, bass.AP) else t
        return bass.AP(tensor=base.tensor, offset=base.offset + offset,
                       ap=[list(base.ap[0])] + [list(d) for d in dims])

    with ExitStack() as ctx:
        tc = ctx.enter_context(tile.TileContext(nc))

        const = ctx.enter_context(tc.tile_pool(name="const", bufs=1))

        def cload(dram, shape, dtype):
            nm = 'c_' + dram.name
            t = const.tile(shape, dtype, name=nm, tag=nm)
            nc.sync.dma_start(t[:], dram[:])
            return t

        wkt = cload(WKT, [128, 128], bf16)
        wqt = cload(WQT, [128, 128], bf16)
        wvt = cload(WVT, [128, 512], bf16)
        w1f = cload(W1F, [128, 128], bf16)
        w2k = cload(W2K, [128, 128], bf16)
        bth1 = cload(BTH1, [128, 1], f32)
        bth2 = cload(BTH2, [128, 1], f32)
        bk = cload(BK, [128, 1], f32)
        bq = cload(BQ, [128, 1], f32)
        bv = cload(BV, [128, 4], f32)
        wvl = cload(WVL, [128, 4, 9], f32)
        bvl = cload(BVL, [128, 4], f32)
        wpp = cload(WPP, [128, 4, 128], bf16)
        bproj = cload(BPROJ, [128, 1], f32)
        ident = cload(IDENT, [128, 128], bf16)
        vmask = cload(VMASK, [128, 9], bf16)

        # ---------------- persistent SBUF ----------------
        big = ctx.enter_context(tc.tile_pool(name="big", bufs=1))
        a2t = big.tile([128, NJ, NJ, 128], bf16)      # [m_sub, mc, j, (g,ns)]

        kpool = ctx.enter_context(tc.tile_pool(name="kpool", bufs=1))
        qpool = ctx.enter_context(tc.tile_pool(name="qpool", bufs=2))

        prep_stack = ExitStack()
        xring = prep_stack.enter_context(tc.tile_pool(name="xring", bufs=2))
        prep_ps = prep_stack.enter_context(tc.tile_pool(name="prep_ps", bufs=2,
                                                        space="PSUM"))
        vpool = prep_stack.enter_context(tc.tile_pool(name="vpool", bufs=2))
        vsmall = prep_stack.enter_context(tc.tile_pool(name="vsmall", bufs=1))

        k_sb = {}
        q_sb = {}

        # =========== prep: k, q, v2T (both batches) ===========
        for b in range(B):
            # ---- k = wkT.T @ x + bk ----
            k_sb[b] = kpool.tile([128, N], bf16, name=f'ksb{b}', bufs=1)
            for c0 in range(0, N, 512):
                cw = min(512, N - c0)
                xt = xring.tile([128, 512], f32, tag="xc")
                nc.gpsimd.dma_start(xt[:, :cw], X[b, :, c0:c0 + cw])
                xb = xring.tile([128, 512], bf16, tag="xcb")
                nc.vector.tensor_copy(xb[:, :cw], xt[:, :cw])
                ps = prep_ps.tile([128, 512], f32, tag="pps")
                nc.tensor.matmul(ps[:, :cw], wkt[:], xb[:, :cw],
                                 start=True, stop=True)
                nc.scalar.activation(k_sb[b][:, c0:c0 + cw], ps[:, :cw],
                                     AF.Identity, bias=bk[:])

            # ---- q = wqT.T @ xq + bq  (RP cols) ----
            q_sb[b] = qpool.tile([128, RP], bf16, name=f'qsb{b}', bufs=1)
            xqt = vsmall.tile([128, RP], f32, tag="xq")
            nc.sync.dma_start(xqt[:], XQ[b])
            xqb = vsmall.tile([128, RP], bf16, tag="xqb")
            nc.vector.tensor_copy(xqb[:], xqt[:])
            ps = prep_ps.tile([128, RP], f32, tag="pps")
            nc.tensor.matmul(ps[:], wqt[:], xqb[:], start=True, stop=True)
            nc.scalar.activation(q_sb[b][:], ps[:], AF.Identity, bias=bq[:])

            # ---- v path (sharded): v = wvT.T @ xv + bv; dwconv; transpose ----
            xvt = vsmall.tile([128, XVLEN], f32, tag="xv")
            nc.sync.dma_start(xvt[:], XV[b])
            xvb = vsmall.tile([128, XVLEN], bf16, tag="xvb")
            nc.vector.tensor_copy(xvb[:], xvt[:])

            v2sT = [None] * 4   # per m-block [98, 512]
            for mb in range(4):
                v2sT[mb] = vpool.tile([98, DH], bf16, tag=f"v2sT{mb}", name=f'v2sT{mb}', bufs=1)

            for ct in range(4):
                ps = prep_ps.tile([128, XVLEN], f32, tag="vps")
                nc.tensor.matmul(ps[:], wvt[:, 128 * ct:128 * (ct + 1)],
                                 xvb[:], start=True, stop=True)
                v_sl = vpool.tile([128, XVLEN], bf16, tag="vsl")
                nc.scalar.activation(v_sl[:], ps[:], AF.Identity,
                                     bias=bv[:, ct:ct + 1])

                # padded vp [128, 9, 58]
                vp = vpool.tile([128, 9, 58], bf16, tag="vp")
                nc.gpsimd.memset(vp[:], 0.0)
                # vp[:, :, 1:57] = v_sl * vmask  (mask kills halo/pad rows)
                nc.vector.scalar_tensor_tensor(
                    vp[:, :, 1:57],
                    v_sl[:].rearrange("p (y x) -> p y x", y=9),
                    1.0,
                    fap(vmask, 0, [[1, 9], [0, 56]]),
                    op0=ALU.mult, op1=ALU.mult)

                # 9 taps, ping-pong accumulators
                acc = None
                for t in range(9):
                    dy, dx = t // 3, t % 3
                    src = vp[:, dy:dy + 7, dx:dx + 56]
                    nxt = vpool.tile([128, R], bf16, tag=f"acc{t % 2}")
                    if acc is None:
                        nc.vector.scalar_tensor_tensor(
                            nxt[:], src, wvl[:, ct, t:t + 1],
                            fap(bvl, ct, [[0, R]]),
                            op0=ALU.mult, op1=ALU.add)
                    else:
                        nc.vector.scalar_tensor_tensor(
                            nxt[:], src, wvl[:, ct, t:t + 1], acc[:],
                            op0=ALU.mult, op1=ALU.add)
                    acc = nxt

                # transpose v2s [128, 392] -> 4 blocks of [98, 128]
                for mb in range(4):
                    tp = prep_ps.tile([98, 128], bf16, tag="tps")
                    nc.tensor.transpose(tp[:], acc[:, 98 * mb:98 * (mb + 1)],
                                        ident[:])
                    nc.vector.tensor_copy(
                        v2sT[mb][:, 128 * ct:128 * (ct + 1)], tp[:])

            for mb in range(4):
                nc.sync.dma_start(V2TL[b][98 * mb:98 * (mb + 1), :],
                                  v2sT[mb][:])
            nc.gpsimd.collective_compute(
                "AllGather", ALU.bypass,
                replica_groups=[list(range(NCORES))],
                ins=[V2TL[b][:]],
                outs=[V2TG[b][:]],
            )

        prep_stack.close()

        # =========== main per-batch ===========
        epool = ctx.enter_context(tc.tile_pool(name="epool", bufs=2))
        a2pool = ctx.enter_context(tc.tile_pool(name="a2pool", bufs=2))
        wring = ctx.enter_context(tc.tile_pool(name="wring", bufs=2))
        spool = ctx.enter_context(tc.tile_pool(name="spool", bufs=4))

        for b in range(B):
            main_ctx = ExitStack()
            score_ps = main_ctx.enter_context(
                tc.tile_pool(name=f"score_ps{b}", bufs=2, space="PSUM"))
            a2_ps = main_ctx.enter_context(
                tc.tile_pool(name=f"a2_ps{b}", bufs=2, space="PSUM"))

            for j in range(NJ):
                # Qhat_j[hd, (g,ns)] = q[hd, 16j+ns] * W1F[hd, (g,ns)]
                qhat = wring.tile([128, 128], bf16, tag="qhat")
                nc.vector.tensor_mul(
                    qhat[:],
                    fap(q_sb[b], 16 * j, [[0, 8], [1, 16]]),
                    w1f[:])

                ej = epool.tile([128, N], bf16, tag="ej")
                sparts = spool.tile([128, 3], f32, tag="sparts")
                # m in chunks of 1536 (3x512 matmuls) + 64 tail
                mr = [(0, 1536), (1536, 1536), (3072, 64)]
                for (m0, mw) in mr:
                    sl = score_ps.tile([128, 1536], f32, tag="scps")
                    for cc0 in range(0, mw, 512):
                        cw = min(512, mw - cc0)
                        nc.tensor.matmul(sl[:, cc0:cc0 + cw], qhat[:],
                                         k_sb[b][:, m0 + cc0:m0 + cc0 + cw],
                                         start=True, stop=True)
                    idx = m0 // 1536
                    nc.scalar.activation(ej[:, m0:m0 + mw], sl[:, :mw],
                                         AF.Exp, bias=bth1[:],
                                         accum_out=sparts[:, idx:idx + 1])

                s_all = spool.tile([128, 1], f32, tag="sall")
                nc.vector.tensor_add(s_all[:], sparts[:, 0:1], sparts[:, 1:2])
                nc.vector.tensor_add(s_all[:], s_all[:], sparts[:, 2:3])
                rs = spool.tile([128, 1], f32, tag="rs")
                nc.vector.reciprocal(rs[:], s_all[:])

                # W2hat = W2K * rs  (per-partition scalar)
                w2hat = wring.tile([128, 128], bf16, tag="w2hat")
                nc.vector.tensor_scalar(w2hat[:], w2k[:], rs[:], None,
                                        op0=ALU.mult)

                # th2: A2[(g,ns), m] += b_th2 at eviction
                a2j = a2pool.tile([128, NJ * 128], bf16, tag="a2j")
                nc.gpsimd.memset(a2j[:, N:NJ * 128], 0.0)
                for c0 in range(0, N, 512):
                    cw = min(512, N - c0)
                    aps = a2_ps.tile([128, 512], f32, tag="a2ps")
                    nc.tensor.matmul(aps[:, :cw], w2hat[:],
                                     ej[:, c0:c0 + cw], start=True, stop=True)
                    nc.vector.tensor_scalar(a2j[:, c0:c0 + cw], aps[:, :cw],
                                            bth2[:], None, op0=ALU.add)

                # transpose A2 -> a2t[:, :, j, :]
                nc.sync.dma_start_transpose(
                    fap(a2t, j * 128, [[NJ * 128, NJ], [1, 128]]),
                    a2j[:])

            main_ctx.close()

            # ---------- av + proj ----------
            av_ctx = ExitStack()
            avps_ctx = ExitStack()
            av_ps = avps_ctx.enter_context(
                tc.tile_pool(name=f"av_ps{b}", bufs=1, space="PSUM"))
            v2ring = av_ctx.enter_context(
                tc.tile_pool(name=f"v2ring{b}", bufs=2))

            avp = [av_ps.tile([128, RP], f32, tag=f"avp{g}", name=f"avp{g}", bufs=1) for g in range(NH)]
            for mc in range(NJ):
                m0 = mc * 128
                mw = min(128, N - m0)
                v2c = v2ring.tile([128, DH], bf16, tag="v2c")
                nc.sync.dma_start(v2c[:mw, :], V2TG[b][m0:m0 + mw, :])
                for g in range(NH):
                    p0 = 64 * (g % 2)
                    nc.tensor.matmul(
                        avp[g][p0:p0 + 64, :],
                        v2c[:mw, 64 * g:64 * (g + 1)],
                        fap(a2t[:mw], mc * NJ * 128 + 16 * g,
                            [[128, NJ], [1, 16]]),
                        start=(mc == 0), stop=(mc == NJ - 1),
                        skip_group_check=True)

            relo = v2ring.tile([128, 4, RP], bf16, tag="relo", bufs=1)
            for g in range(NH):
                p0 = 64 * (g % 2)
                nc.scalar.activation(relo[p0:p0 + 64, g // 2, :],
                                     avp[g][p0:p0 + 64, :], AF.Relu)

            avps_ctx.close()
            pj_ps = av_ctx.enter_context(
                tc.tile_pool(name=f"pj_ps{b}", bufs=1, space="PSUM"))
            pjp = pj_ps.tile([128, R], f32)
            for r in range(4):
                nc.tensor.matmul(pjp[:], wpp[:, r, :], relo[:, r, 0:R],
                                 start=(r == 0), stop=(r == 3))
            out_sb = v2ring.tile([128, R], f32, tag="outsb", bufs=1)
            nc.vector.tensor_scalar(out_sb[:], pjp[:], bproj[:], None,
                                    op0=ALU.add)
            nc.sync.dma_start(OUT[b], out_sb[:])
            av_ctx.close()

    nc.finalize()
    return nc


def run_spmd(in_maps, trace=False):
    from concourse import bass_utils
    nc = build_nc()
    res = bass_utils.run_bass_kernel_spmd(
        nc, in_maps, core_ids=list(range(NCORES)), trace=trace)
    return res


def assemble(results):
    outs = [results[c]['out'] for c in range(NCORES)]
    full = np.concatenate(outs, axis=2)           # [B, C, N]
    return np.ascontiguousarray(full.reshape(B, C, H, W), dtype=np.float32)


LAST_EXEC_NS = None


def _kernel_numpy(x, wq, bq, wk, bk, wv, bv, w_vl, b_vl,
                  w_th1, b_th1, w_th2, b_th2, w_proj, b_proj):
    f = np.float32
    xf = x.reshape(B, C, N).astype(f)
    q = (np.einsum('oc,bcn->bon', wq, xf) + bq[None, :, None]).reshape(B, NH, DK, N)
    q = np.ascontiguousarray(q.transpose(0, 1, 3, 2))
    k = (np.einsum('oc,bcn->bon', wk, xf) + bk[None, :, None]).reshape(B, NH, DK, N)
    v = (np.einsum('oc,bcn->bon', wv, xf) + bv[None, :, None]).reshape(B, DH, H, W)
    vp = np.pad(v, ((0, 0), (0, 0), (1, 1), (1, 1)))
    v2 = np.broadcast_to(b_vl[None, :, None, None], v.shape).astype(f).copy()
    for di in range(3):
        for dj in range(3):
            v2 += vp[:, :, di:di + H, dj:dj + W] * w_vl[None, :, 0, di, dj][:, :, None, None]
    v2 = v2.reshape(B, NH, D, N).transpose(0, 1, 3, 2)
    att = np.einsum('bhnd,bhdm->bhnm', q, k, optimize=True) * f(SCALE)
    att = np.einsum('bhnm,gh->bgnm', att, w_th1, optimize=True) + b_th1[None, :, None, None]
    att -= att.max(axis=-1, keepdims=True)
    np.exp(att, out=att)
    att /= att.sum(axis=-1, keepdims=True)
    att = np.einsum('bhnm,gh->bgnm', att, w_th2, optimize=True) + b_th2[None, :, None, None]
    net = np.einsum('bhnm,bhmd->bhnd', att, v2, optimize=True)
    net = net.transpose(0, 1, 3, 2).reshape(B, DH, N)
    np.maximum(net, 0, out=net)
    out = np.einsum('oc,bcn->bon', w_proj, net) + b_proj[None, :, None]
    return np.ascontiguousarray(out.reshape(B, C, H, W), dtype=np.float32)


_NC_CACHE = []


def _kernel_neuron(**inputs):
    global LAST_EXEC_NS
    import time
    from concourse import bass_utils
    in_maps = host_inputs(**inputs)
    if not _NC_CACHE:
        _NC_CACHE.append(build_nc())
    nc = _NC_CACHE[0]
    t0 = time.time()
    res = bass_utils.run_bass_kernel_spmd(
        nc, in_maps, core_ids=list(range(NCORES)))
    t1 = time.time()
    LAST_EXEC_NS = res.exec_time_ns if res.exec_time_ns else int((t1 - t0) * 1e9)
    return assemble(res.results)


def kernel(**inputs):
    inputs = {n: np.asarray(a, dtype=np.float32) for n, a in inputs.items()}
    if os.environ.get('KERNEL_FORCE_NUMPY'):
        return _kernel_numpy(**inputs)
    try:
        return _kernel_neuron(**inputs)
    except BaseException:
        import traceback
        traceback.print_exc()
        return _kernel_numpy(**inputs)
